# revision 20
# baseline (speedup 1.0000x reference)
"""CGRU cell on 8 Trainium2 NeuronCores.

Strategy: data-parallel over the batch dim (4096 -> 8 x 512). Each core
computes its h-shard with zero cross-core communication; weights are
replicated.

On-core compute runs in transposed space ([feature, batch]): the gate
pre-activations are (x @ W)^T = W^T @ x^T, so W tiles are the stationary
matmul operand and x^T/h^T tiles [128, 512] are the moving operand
(batch = 512 = one full fp32 PSUM bank).

The complex product uses Karatsuba (3 real matmuls instead of 4):
  real = k1 + xi@(I-R),  imag = k1 - xr@(R+I),  k1 = (xr+xi)@R
so each (gate, m-tile) job runs three 16-matmul PSUM chains (k1 /
real-extra / imag-extra over the input and recurrent halves) = 48
matmuls instead of the naive 64 -- a 25% cut in PE streaming time,
which is the bottleneck.  The combined weights R+I and I-R are built
on-chip from the streamed R, I tiles with one scalar_tensor_tensor
each, so weight DMA stays at 4 tiles/job.  The activation sums xr+xi,
hr+hi (and clip(r)*h sums for the candidate gate) are computed on-chip
on the vector engine.  Matmuls are fp16 with fp32 PSUM accumulation;
PSUM pairs are combined (k1 + extra) on the vector engine, activations
on the scalar engine.

DMA queues: weights stream on the sync queue, activations load on the
gpsimd queue, outputs store (fp16, upcast on host) on the gpsimd queue.
"""

import numpy as np

import concourse.bass as bass
import concourse.mybir as mybir
import concourse.tile as tile
from concourse import bacc
from concourse.bass_utils import run_bass_kernel_spmd

B, D, U = 4096, 1024, 1024
NCORES = 8
N = B // NCORES          # batch rows per core (moving free dim)
P = 128                  # partition size
KT = D // P              # 8 k-tiles per complex half
MT = U // P              # 8 m-tiles per complex half
F = 2 * U                # 2048 features
MCOLS = KT * P           # 1024 cols per per-matrix weight tile

F16 = mybir.dt.float16
F32 = mybir.dt.float32
AF = mybir.ActivationFunctionType
OP = mybir.AluOpType

_CACHE = {}


def _build():
    nc = bacc.Bacc("TRN2", target_bir_lowering=False, debug=False)

    xT = nc.dram_tensor("xT", [F, N], F16, kind="ExternalInput")
    hT = nc.dram_tensor("hT", [F, N], F16, kind="ExternalInput")
    w1 = nc.dram_tensor("w1", [MT, 2, 4, P, MCOLS], F16, kind="ExternalInput")
    w2 = nc.dram_tensor("w2", [MT, 4, P, MCOLS], F16, kind="ExternalInput")
    bzr = nc.dram_tensor("bzr", [P, 2, 2 * MT], F32, kind="ExternalInput")
    bh = nc.dram_tensor("bh", [P, 2 * MT], F32, kind="ExternalInput")
    oT = nc.dram_tensor("oT", [F, N], F16, kind="ExternalOutput")

    with tile.TileContext(nc) as tc:
        with (
            tc.tile_pool(name="res", bufs=1) as res,
            tc.tile_pool(name="wts", bufs=24) as wts,
            tc.tile_pool(name="act", bufs=10) as act,
            tc.tile_pool(name="ps", bufs=8, space="PSUM") as psp,
        ):
            # PE warmup: dummy matmuls on a zeroed tile keep the HAM
            # activity window busy while the first real DMAs land.
            wsrc = res.tile([P, P], F16, tag="wsrc")
            dmov = res.tile([P, N], F16, tag="dmov")
            nc.vector.memset(wsrc[:], 0.0)
            nc.vector.memset(dmov[:], 0.0)
            wps = psp.tile([P, N], F32, tag="ps")
            for _ in range(8):
                nc.tensor.matmul(wps[:], wsrc[:], dmov[:], start=True, stop=True)

            xs = res.tile([P, 2 * MT, N], F16, tag="xs")
            xsm = res.tile([P, MT, N], F16, tag="xsm")
            hs = res.tile([P, 2 * MT, N], F16, tag="hs")
            hsm = res.tile([P, MT, N], F16, tag="hsm")
            zs = res.tile([P, 2 * MT, N], F16, tag="zs")
            rh = res.tile([P, 2 * MT, N], F16, tag="rh")
            rhsm = res.tile([P, MT, N], F16, tag="rhsm")
            zh = res.tile([P, 2, N], F16, tag="zh")
            mu = res.tile([P, 2, N], F16, tag="mu")
            ones = res.tile([P, N], F16, tag="ones")
            nc.vector.memset(ones[:], 1.0)
            bz_sb = res.tile([P, 2, 2 * MT], F32, tag="bz")
            bh_sb = res.tile([P, 2 * MT], F32, tag="bh")

            nc.scalar.dma_start(bz_sb[:], bzr[:])
            nc.scalar.dma_start(bh_sb[:], bh[:])

            def load_pair(src_r, src_i):
                """Stream R, I; build the Karatsuba combos on the DVE."""
                wr = wts.tile([P, MCOLS], F16, tag="w")
                nc.sync.dma_start(wr[:], src_r)
                wi = wts.tile([P, MCOLS], F16, tag="w")
                nc.sync.dma_start(wi[:], src_i)
                w2t = wts.tile([P, MCOLS], F16, tag="w")    # R+I
                nc.vector.tensor_tensor(w2t[:], wr[:], wi[:], OP.add)
                w3t = wts.tile([P, MCOLS], F16, tag="w")    # I-R
                nc.vector.tensor_tensor(w3t[:], wi[:], wr[:], OP.subtract)
                return wr, w2t, w3t

            def stream_acts(src_dram, dst, dst_sum):
                """(real, imag) pairs in consumption order; the on-chip
                sum follows each pair so the k1 chain is fed at arrival
                rate."""
                for k in range(MT):
                    nc.gpsimd.dma_start(dst[:, k, :],
                                        src_dram[k * P:(k + 1) * P, :])
                    nc.gpsimd.dma_start(dst[:, MT + k, :],
                                        src_dram[(MT + k) * P:(MT + k + 1) * P, :])
                    nc.vector.tensor_tensor(dst_sum[:, k, :], dst[:, k, :],
                                            dst[:, MT + k, :], OP.add)

            def accum_side(w_, movs, pss, side, kmajor=False,
                           cols=slice(0, N)):
                """24 matmuls: one (input or recurrent) side of a job.

                kmajor interleaves the three chains per k-tile so PE
                consumption of freshly-DMA'd moving tiles matches
                arrival order (xr, xi, then the DVE'd sum)."""
                wr_, w2_, w3_ = w_
                msum, mr, mi = movs
                psK, psR, psI = pss
                # psK first: its chain stops 16 MMs before the job ends,
                # so the k1 copy + real-half combine overlap the psR/psI
                # chains and only the imag half trails the last matmul
                triples = ((psK, wr_, msum), (psR, w3_, mi), (psI, w2_, mr))

                def mm(ps, w, mv, k):
                    nc.tensor.matmul(ps[:, cols], w[:, k * P:(k + 1) * P],
                                     mv(k)[:, cols],
                                     start=(side == 0 and k == 0),
                                     stop=(side == 1 and k == KT - 1))

                if kmajor:
                    for k in range(KT):
                        for ps, w, mv in triples:
                            mm(ps, w, mv, k)
                else:
                    for ps, w, mv in triples:
                        for k in range(KT):
                            mm(ps, w, mv, k)

            def xr(k):
                return xs[:, k, :]

            def xi(k):
                return xs[:, MT + k, :]

            def xsu(k):
                return xsm[:, k, :]

            def hr(k):
                return hs[:, k, :]

            def hi(k):
                return hs[:, MT + k, :]

            def hsu(k):
                return hsm[:, k, :]

            def rhr(k):
                return rh[:, k, :]

            def rhi(k):
                return rh[:, MT + k, :]

            def rhsu(k):
                return rhsm[:, k, :]

            movx = (xsu, xr, xi)
            movh = (hsu, hr, hi)
            movrh = (rhsu, rhr, rhi)

            def combine_zr(g, m, pss):
                psK, psR, psI = pss
                # DVE can read only one PSUM input per instruction, so the
                # shared k1 term goes through SBUF once (on the scalar eng)
                k1s = act.tile([P, N], F16, tag="k1s")
                nc.scalar.activation(k1s[:], psK[:], AF.Copy)
                for half, psX in ((0, psR), (1, psI)):
                    mm = m if half == 0 else MT + m
                    d = act.tile([P, N], F16, tag="d")
                    nc.vector.tensor_tensor(d[:], k1s[:], psX[:],
                                            OP.add if half == 0 else OP.subtract)
                    if g == 0:
                        nc.scalar.activation(zs[:, mm, :], d[:], AF.Relu,
                                             bias=bz_sb[:, 0, mm:mm + 1],
                                             scale=0.2)
                    else:
                        rr = act.tile([P, N], F16, tag="rr")
                        nc.scalar.activation(rr[:], d[:], AF.Relu,
                                             bias=bz_sb[:, 1, mm:mm + 1],
                                             scale=0.2)
                        nc.vector.scalar_tensor_tensor(
                            rh[:, mm, :], rr[:], 1.0, hs[:, mm, :],
                            op0=OP.min, op1=OP.mult)
                if g == 1:
                    nc.vector.tensor_tensor(rhsm[:, m, :], rh[:, m, :],
                                            rh[:, MT + m, :], OP.add)

            def combine_h(m, pss, cols):
                psK, psR, psI = pss
                k1s = act.tile([P, N], F16, tag="k1s")
                nc.scalar.activation(k1s[:, cols], psK[:, cols], AF.Copy)
                for half, psX in ((0, psR), (1, psI)):
                    mm = m if half == 0 else MT + m
                    d = act.tile([P, N], F16, tag="d")
                    nc.vector.tensor_tensor(d[:, cols], k1s[:, cols],
                                            psX[:, cols],
                                            OP.add if half == 0 else OP.subtract)
                    t = act.tile([P, N], F16, tag="t")
                    nc.scalar.activation(t[:, cols], d[:, cols], AF.Tanh,
                                         bias=bh_sb[:, mm:mm + 1])
                    e = act.tile([P, N], F16, tag="d")
                    nc.vector.scalar_tensor_tensor(
                        e[:, cols], t[:, cols], -1.0, hs[:, mm, cols],
                        op0=OP.mult, op1=OP.add)          # h - t
                    f = act.tile([P, N], F16, tag="d")
                    nc.vector.scalar_tensor_tensor(
                        f[:, cols], zs[:, mm, cols], 1.0, e[:, cols],
                        op0=OP.min, op1=OP.mult)          # min(z,1)*(h-t)
                    o = act.tile([P, N], F16, tag="o")
                    nc.vector.tensor_tensor(o[:, cols], f[:, cols],
                                            t[:, cols], OP.add)
                    nc.gpsimd.dma_start(oT[mm * P:(mm + 1) * P, cols],
                                        o[:, cols])

            def combine_h_last(m, pss, cols):
                """Tail-optimized combine: zh = min(z,1)*h and
                mu = min(z,1)-1 are precomputed, so only two DVE ops
                trail the tanh:  o = zh - mu*t."""
                psK, psR, psI = pss
                k1s = act.tile([P, N], F16, tag="k1s")
                nc.scalar.activation(k1s[:, cols], psK[:, cols], AF.Copy)
                for half, psX in ((0, psR), (1, psI)):
                    mm = m if half == 0 else MT + m
                    d = act.tile([P, N], F16, tag="d")
                    nc.vector.tensor_tensor(d[:, cols], k1s[:, cols],
                                            psX[:, cols],
                                            OP.add if half == 0 else OP.subtract)
                    t = act.tile([P, N], F16, tag="t")
                    nc.scalar.activation(t[:, cols], d[:, cols], AF.Tanh,
                                         bias=bh_sb[:, mm:mm + 1])
                    v = act.tile([P, N], F16, tag="d")
                    nc.vector.tensor_tensor(v[:, cols], mu[:, half, cols],
                                            t[:, cols], OP.mult)
                    o = act.tile([P, N], F16, tag="o")
                    nc.vector.tensor_tensor(o[:, cols], zh[:, half, cols],
                                            v[:, cols], OP.subtract)
                    nc.gpsimd.dma_start(oT[mm * P:(mm + 1) * P, cols],
                                        o[:, cols])

            def job_psums():
                return tuple(psp.tile([P, N], F32, tag="ps", name=f"ps{i}")
                             for i in range(3))

            def accum_fused(wjobs, movs, side):
                """Both m=0 jobs' chains interleaved per k-tile: each
                freshly-DMA'd moving tile feeds 6 matmuls back-to-back,
                halving the arrival bandwidth the PE needs during the
                HBM-bound startup."""
                msum, mr, mi = movs
                # w_ is ordered (wr, w3, w2) to match pss (psK, psR, psI):
                # psI<-w2 consumes mr, psR<-w3 consumes mi, psK<-wr the sum
                for k in range(KT):
                    for sel, mv in ((2, mr), (1, mi), (0, msum)):
                        for w_, pss in wjobs:
                            nc.tensor.matmul(
                                pss[sel][:], w_[sel][:, k * P:(k + 1) * P],
                                mv(k)[:],
                                start=(side == 0 and k == 0),
                                stop=(side == 1 and k == KT - 1))

            # startup emission order: m=0 x-side weights (sync DMA +
            # DVE combos) FIRST so the combos are not stuck in the DVE
            # FIFO behind h-dependent sum ops, then the x stream, the
            # rec weights, and last the h stream.  The two wr tiles load
            # before the wi tiles so the psK chains (which need only wr
            # and the x sums) can start ~2.5us in, before the combos.
            wrs0, wis0, wx0 = [], [], []
            for g in range(2):
                wr = wts.tile([P, MCOLS], F16, tag="w", name=f"wr0{g}")
                nc.sync.dma_start(wr[:], w1[0, g, 0])
                wrs0.append(wr)
            for g in range(2):
                wi = wts.tile([P, MCOLS], F16, tag="w", name=f"wi0{g}")
                nc.sync.dma_start(wi[:], w1[0, g, 1])
                wis0.append(wi)
            for g in range(2):
                w2t = wts.tile([P, MCOLS], F16, tag="w", name=f"w2t0{g}")
                nc.vector.tensor_tensor(w2t[:], wrs0[g][:], wis0[g][:], OP.add)
                w3t = wts.tile([P, MCOLS], F16, tag="w", name=f"w3t0{g}")
                nc.vector.tensor_tensor(w3t[:], wis0[g][:], wrs0[g][:],
                                        OP.subtract)
                wx0.append((wrs0[g], w2t, w3t))
            stream_acts(xT, xs, xsm)
            wrecs0 = [load_pair(w1[0, g, 2], w1[0, g, 3]) for g in range(2)]
            stream_acts(hT, hs, hsm)
            w2m0 = None

            # ---- phase 1: z and r gates, rh = clip(r)*h ----
            for m in range(MT):
                if m == MT - 1:
                    # hoist phase-2 m=0 weights ahead of the phase-1 tail
                    # so its combos clear the DVE before the phase switch
                    w2m0 = (load_pair(w2[0, 0], w2[0, 1]),
                            load_pair(w2[0, 2], w2[0, 3]))
                if m == 0:
                    jobs = [(g, wx0[g], job_psums()) for g in range(2)]
                    # x-side chain-major: psK chains only need wr + xsum
                    # (first bytes to arrive); psR/psI run once the wi
                    # tiles and DVE combos land
                    for g, sel, mv in ((0, 0, xsu), (1, 0, xsu),
                                       (0, 1, xi), (0, 2, xr),
                                       (1, 1, xi), (1, 2, xr)):
                        _, wx, pss = jobs[g]
                        w_ = (wx[0], wx[2], wx[1])
                        for k in range(KT):
                            nc.tensor.matmul(
                                pss[sel][:],
                                w_[sel][:, k * P:(k + 1) * P], mv(k)[:],
                                start=(k == 0), stop=False)
                    accum_fused([((wrecs0[g][0], wrecs0[g][2], wrecs0[g][1]),
                                  pss) for g, wx, pss in jobs], movh, 1)
                    for g, wx, pss in jobs:
                        combine_zr(g, m, pss)
                else:
                    for g in range(2):
                        wx = load_pair(w1[m, g, 0], w1[m, g, 1])
                        wrec = load_pair(w1[m, g, 2], w1[m, g, 3])
                        pss = job_psums()
                        accum_side(wx, movx, pss, 0)
                        accum_side(wrec, movh, pss, 1)
                        combine_zr(g, m, pss)

            # tail prep: zh/mu for the last m-tile (runs in late phase 1)
            for half in range(2):
                mm = (MT - 1) + half * MT
                nc.vector.scalar_tensor_tensor(
                    zh[:, half, :], zs[:, mm, :], 1.0, hs[:, mm, :],
                    op0=OP.min, op1=OP.mult)              # min(z,1)*h
                nc.vector.scalar_tensor_tensor(
                    mu[:, half, :], zs[:, mm, :], 1.0, ones[:],
                    op0=OP.min, op1=OP.subtract)          # min(z,1)-1

            # ---- phase 2: hh gate + final combine ----
            for m in range(MT):
                if m == 0:
                    wx, wrec = w2m0
                else:
                    wx = load_pair(w2[m, 0], w2[m, 1])
                    wrec = load_pair(w2[m, 2], w2[m, 3])
                pss = job_psums()
                if m == MT - 1:
                    # last job in column halves so only a half-width
                    # combine chain trails the final matmul
                    for cols in (slice(0, N // 2), slice(N // 2, N)):
                        accum_side(wx, movx, pss, 0, cols=cols)
                        accum_side(wrec, movrh, pss, 1, cols=cols)
                        combine_h_last(m, pss, cols)
                else:
                    accum_side(wx, movx, pss, 0)
                    accum_side(wrec, movrh, pss, 1)
                    combine_h(m, pss, slice(0, N))

    nc.compile()
    return nc


def _tiles(mat):
    # (1024, 1024) -> [p, k, 128, 128] tile array
    return mat.reshape(KT, P, MT, P).transpose(2, 0, 1, 3)


def _gate_blob(mats):
    """[p, 4, 128, MCOLS] fp16 weight blob from (R, I, RR, IR)."""
    arr = np.stack([_tiles(m) for m in mats])  # [4, p, k, part, col]
    arr = arr.transpose(1, 0, 3, 2, 4)         # [p, mat, part, k, col]
    return arr.reshape(MT, 4, P, MCOLS).astype(np.float16)


def prepare_in_maps(inputs, h_tm1, real_kernel, imaginary_kernel,
                    real_recurrent_kernel, imaginary_recurrent_kernel,
                    real_bias, imaginary_bias):
    inputs = np.asarray(inputs, dtype=np.float32)
    h_tm1 = np.asarray(h_tm1, dtype=np.float32)

    def gate(Wmat, g):
        return np.asarray(Wmat[:, g * U:(g + 1) * U], dtype=np.float32)

    def mats(g):
        R, I = gate(real_kernel, g), gate(imaginary_kernel, g)
        RR, IR = gate(real_recurrent_kernel, g), gate(imaginary_recurrent_kernel, g)
        return (R, I, RR, IR)

    w1_np = np.ascontiguousarray(
        np.stack([_gate_blob(mats(0)), _gate_blob(mats(1))], axis=1))
    w2_np = np.ascontiguousarray(_gate_blob(mats(2)))

    def cat_bias(g):
        return np.concatenate([
            np.asarray(real_bias[g * U:(g + 1) * U], dtype=np.float32),
            np.asarray(imaginary_bias[g * U:(g + 1) * U], dtype=np.float32),
        ])

    bzr_np = np.ascontiguousarray(np.stack(
        [0.2 * cat_bias(g) + 0.5 for g in range(2)]).reshape(2, 2 * MT, P)
        .transpose(2, 0, 1))
    bh_np = np.ascontiguousarray(cat_bias(2).reshape(2 * MT, P).T)

    in_maps = []
    for c in range(NCORES):
        sl = slice(c * N, (c + 1) * N)
        in_maps.append({
            "xT": inputs[sl].T.astype(np.float16),
            "hT": h_tm1[sl].T.astype(np.float16),
            "w1": w1_np, "w2": w2_np, "bzr": bzr_np, "bh": bh_np,
        })
    return in_maps


def get_nc():
    if "nc" not in _CACHE:
        _CACHE["nc"] = _build()
    return _CACHE["nc"]


def gather(results):
    out = np.empty((B, F), dtype=np.float32)
    for c in range(NCORES):
        out[c * N:(c + 1) * N] = res_oT(results, c)
    return out


def res_oT(results, c):
    return results[c]["oT"].T


def kernel(**inputs):
    nc = get_nc()
    in_maps = prepare_in_maps(**inputs)
    res = run_bass_kernel_spmd(nc, in_maps, list(range(NCORES)))
    return gather(res.results)


# revision 21
# speedup vs baseline: 1.0180x; 1.0180x over previous
"""CGRU cell on 8 Trainium2 NeuronCores.

Strategy: data-parallel over the batch dim (4096 -> 8 x 512). Each core
computes its h-shard with zero cross-core communication; weights are
replicated.

On-core compute runs in transposed space ([feature, batch]): the gate
pre-activations are (x @ W)^T = W^T @ x^T, so W tiles are the stationary
matmul operand and x^T/h^T tiles [128, 512] are the moving operand
(batch = 512 = one full fp32 PSUM bank).

The complex product uses Karatsuba (3 real matmuls instead of 4):
  real = k1 + xi@(I-R),  imag = k1 - xr@(R+I),  k1 = (xr+xi)@R
so each (gate, m-tile) job runs three 16-matmul PSUM chains (k1 /
real-extra / imag-extra over the input and recurrent halves) = 48
matmuls instead of the naive 64 -- a 25% cut in PE streaming time,
which is the bottleneck.  The combined weights R+I and I-R are built
on-chip from the streamed R, I tiles with one scalar_tensor_tensor
each, so weight DMA stays at 4 tiles/job.  The activation sums xr+xi,
hr+hi (and clip(r)*h sums for the candidate gate) are computed on-chip
on the vector engine.  Matmuls are fp16 with fp32 PSUM accumulation;
PSUM pairs are combined (k1 + extra) on the vector engine, activations
on the scalar engine.

DMA queues: weights stream on the sync queue, activations load on the
gpsimd queue, outputs store (fp16, upcast on host) on the sync queue.
"""

import numpy as np

import concourse.bass as bass
import concourse.mybir as mybir
import concourse.tile as tile
from concourse import bacc
from concourse.bass_utils import run_bass_kernel_spmd

B, D, U = 4096, 1024, 1024
NCORES = 8
N = B // NCORES          # batch rows per core (moving free dim)
P = 128                  # partition size
KT = D // P              # 8 k-tiles per complex half
MT = U // P              # 8 m-tiles per complex half
F = 2 * U                # 2048 features
MCOLS = KT * P           # 1024 cols per per-matrix weight tile

F16 = mybir.dt.float16
F32 = mybir.dt.float32
AF = mybir.ActivationFunctionType
OP = mybir.AluOpType

_CACHE = {}


def _build():
    nc = bacc.Bacc("TRN2", target_bir_lowering=False, debug=False)

    xT = nc.dram_tensor("xT", [F, N], F16, kind="ExternalInput")
    hT = nc.dram_tensor("hT", [F, N], F16, kind="ExternalInput")
    w1 = nc.dram_tensor("w1", [MT, 2, 4, P, MCOLS], F16, kind="ExternalInput")
    w2 = nc.dram_tensor("w2", [MT, 4, P, MCOLS], F16, kind="ExternalInput")
    bzr = nc.dram_tensor("bzr", [P, 2, 2 * MT], F32, kind="ExternalInput")
    bh = nc.dram_tensor("bh", [P, 2 * MT], F32, kind="ExternalInput")
    oT = nc.dram_tensor("oT", [F, N], F16, kind="ExternalOutput")

    with tile.TileContext(nc) as tc:
        with (
            tc.tile_pool(name="res", bufs=1) as res,
            tc.tile_pool(name="wts", bufs=24) as wts,
            tc.tile_pool(name="act", bufs=10) as act,
            tc.tile_pool(name="ps", bufs=8, space="PSUM") as psp,
        ):
            # PE warmup: dummy matmuls on a zeroed tile keep the HAM
            # activity window busy while the first real DMAs land.
            wsrc = res.tile([P, P], F16, tag="wsrc")
            dmov = res.tile([P, N], F16, tag="dmov")
            nc.vector.memset(wsrc[:], 0.0)
            nc.vector.memset(dmov[:], 0.0)
            wps = psp.tile([P, N], F32, tag="ps")
            for _ in range(8):
                nc.tensor.matmul(wps[:], wsrc[:], dmov[:], start=True, stop=True)

            xs = res.tile([P, 2 * MT, N], F16, tag="xs")
            xsm = res.tile([P, MT, N], F16, tag="xsm")
            hs = res.tile([P, 2 * MT, N], F16, tag="hs")
            hsm = res.tile([P, MT, N], F16, tag="hsm")
            zs = res.tile([P, 2 * MT, N], F16, tag="zs")
            rh = res.tile([P, 2 * MT, N], F16, tag="rh")
            rhsm = res.tile([P, MT, N], F16, tag="rhsm")
            zh = res.tile([P, 2, N], F16, tag="zh")
            mu = res.tile([P, 2, N], F16, tag="mu")
            ones = res.tile([P, N], F16, tag="ones")
            nc.vector.memset(ones[:], 1.0)
            bz_sb = res.tile([P, 2, 2 * MT], F32, tag="bz")
            bh_sb = res.tile([P, 2 * MT], F32, tag="bh")

            nc.scalar.dma_start(bz_sb[:], bzr[:])
            nc.scalar.dma_start(bh_sb[:], bh[:])

            def load_pair(src_r, src_i):
                """Stream R, I; build the Karatsuba combos on the DVE."""
                wr = wts.tile([P, MCOLS], F16, tag="w")
                nc.sync.dma_start(wr[:], src_r)
                wi = wts.tile([P, MCOLS], F16, tag="w")
                nc.sync.dma_start(wi[:], src_i)
                w2t = wts.tile([P, MCOLS], F16, tag="w")    # R+I
                nc.vector.tensor_tensor(w2t[:], wr[:], wi[:], OP.add)
                w3t = wts.tile([P, MCOLS], F16, tag="w")    # I-R
                nc.vector.tensor_tensor(w3t[:], wi[:], wr[:], OP.subtract)
                return wr, w2t, w3t

            def stream_acts(src_dram, dst, dst_sum):
                """(real, imag) pairs in consumption order; the on-chip
                sum follows each pair so the k1 chain is fed at arrival
                rate."""
                for k in range(MT):
                    nc.gpsimd.dma_start(dst[:, k, :],
                                        src_dram[k * P:(k + 1) * P, :])
                    nc.gpsimd.dma_start(dst[:, MT + k, :],
                                        src_dram[(MT + k) * P:(MT + k + 1) * P, :])
                    nc.vector.tensor_tensor(dst_sum[:, k, :], dst[:, k, :],
                                            dst[:, MT + k, :], OP.add)

            def accum_side(w_, movs, pss, side, kmajor=False,
                           cols=slice(0, N)):
                """24 matmuls: one (input or recurrent) side of a job.

                kmajor interleaves the three chains per k-tile so PE
                consumption of freshly-DMA'd moving tiles matches
                arrival order (xr, xi, then the DVE'd sum)."""
                wr_, w2_, w3_ = w_
                msum, mr, mi = movs
                psK, psR, psI = pss
                triples = ((psI, w2_, mr), (psR, w3_, mi), (psK, wr_, msum))

                def mm(ps, w, mv, k):
                    nc.tensor.matmul(ps[:, cols], w[:, k * P:(k + 1) * P],
                                     mv(k)[:, cols],
                                     start=(side == 0 and k == 0),
                                     stop=(side == 1 and k == KT - 1))

                if kmajor:
                    for k in range(KT):
                        for ps, w, mv in triples:
                            mm(ps, w, mv, k)
                else:
                    for ps, w, mv in triples:
                        for k in range(KT):
                            mm(ps, w, mv, k)

            def xr(k):
                return xs[:, k, :]

            def xi(k):
                return xs[:, MT + k, :]

            def xsu(k):
                return xsm[:, k, :]

            def hr(k):
                return hs[:, k, :]

            def hi(k):
                return hs[:, MT + k, :]

            def hsu(k):
                return hsm[:, k, :]

            def rhr(k):
                return rh[:, k, :]

            def rhi(k):
                return rh[:, MT + k, :]

            def rhsu(k):
                return rhsm[:, k, :]

            movx = (xsu, xr, xi)
            movh = (hsu, hr, hi)
            movrh = (rhsu, rhr, rhi)

            def combine_zr(g, m, pss):
                psK, psR, psI = pss
                # DVE can read only one PSUM input per instruction, so the
                # shared k1 term goes through SBUF once (on the scalar eng)
                k1s = act.tile([P, N], F16, tag="k1s")
                nc.scalar.activation(k1s[:], psK[:], AF.Copy)
                for half, psX in ((0, psR), (1, psI)):
                    mm = m if half == 0 else MT + m
                    d = act.tile([P, N], F16, tag="d")
                    nc.vector.tensor_tensor(d[:], k1s[:], psX[:],
                                            OP.add if half == 0 else OP.subtract)
                    if g == 0:
                        nc.scalar.activation(zs[:, mm, :], d[:], AF.Relu,
                                             bias=bz_sb[:, 0, mm:mm + 1],
                                             scale=0.2)
                    else:
                        rr = act.tile([P, N], F16, tag="rr")
                        nc.scalar.activation(rr[:], d[:], AF.Relu,
                                             bias=bz_sb[:, 1, mm:mm + 1],
                                             scale=0.2)
                        nc.vector.scalar_tensor_tensor(
                            rh[:, mm, :], rr[:], 1.0, hs[:, mm, :],
                            op0=OP.min, op1=OP.mult)
                if g == 1:
                    nc.vector.tensor_tensor(rhsm[:, m, :], rh[:, m, :],
                                            rh[:, MT + m, :], OP.add)

            def combine_h(m, pss, cols):
                psK, psR, psI = pss
                k1s = act.tile([P, N], F16, tag="k1s")
                nc.scalar.activation(k1s[:, cols], psK[:, cols], AF.Copy)
                for half, psX in ((0, psR), (1, psI)):
                    mm = m if half == 0 else MT + m
                    d = act.tile([P, N], F16, tag="d")
                    nc.vector.tensor_tensor(d[:, cols], k1s[:, cols],
                                            psX[:, cols],
                                            OP.add if half == 0 else OP.subtract)
                    t = act.tile([P, N], F16, tag="t")
                    nc.scalar.activation(t[:, cols], d[:, cols], AF.Tanh,
                                         bias=bh_sb[:, mm:mm + 1])
                    e = act.tile([P, N], F16, tag="d")
                    nc.vector.scalar_tensor_tensor(
                        e[:, cols], t[:, cols], -1.0, hs[:, mm, cols],
                        op0=OP.mult, op1=OP.add)          # h - t
                    f = act.tile([P, N], F16, tag="d")
                    nc.vector.scalar_tensor_tensor(
                        f[:, cols], zs[:, mm, cols], 1.0, e[:, cols],
                        op0=OP.min, op1=OP.mult)          # min(z,1)*(h-t)
                    o = act.tile([P, N], F16, tag="o")
                    nc.vector.tensor_tensor(o[:, cols], f[:, cols],
                                            t[:, cols], OP.add)
                    nc.sync.dma_start(oT[mm * P:(mm + 1) * P, cols],
                                        o[:, cols])

            def combine_h_last(m, pss, cols):
                """Tail-optimized combine: zh = min(z,1)*h and
                mu = min(z,1)-1 are precomputed, so only two DVE ops
                trail the tanh:  o = zh - mu*t."""
                psK, psR, psI = pss
                k1s = act.tile([P, N], F16, tag="k1s")
                nc.scalar.activation(k1s[:, cols], psK[:, cols], AF.Copy)
                for half, psX in ((0, psR), (1, psI)):
                    mm = m if half == 0 else MT + m
                    d = act.tile([P, N], F16, tag="d")
                    nc.vector.tensor_tensor(d[:, cols], k1s[:, cols],
                                            psX[:, cols],
                                            OP.add if half == 0 else OP.subtract)
                    t = act.tile([P, N], F16, tag="t")
                    nc.scalar.activation(t[:, cols], d[:, cols], AF.Tanh,
                                         bias=bh_sb[:, mm:mm + 1])
                    v = act.tile([P, N], F16, tag="d")
                    nc.vector.tensor_tensor(v[:, cols], mu[:, half, cols],
                                            t[:, cols], OP.mult)
                    o = act.tile([P, N], F16, tag="o")
                    nc.vector.tensor_tensor(o[:, cols], zh[:, half, cols],
                                            v[:, cols], OP.subtract)
                    nc.sync.dma_start(oT[mm * P:(mm + 1) * P, cols],
                                        o[:, cols])

            def job_psums():
                return tuple(psp.tile([P, N], F32, tag="ps", name=f"ps{i}")
                             for i in range(3))

            def accum_fused(wjobs, movs, side):
                """Both m=0 jobs' chains interleaved per k-tile: each
                freshly-DMA'd moving tile feeds 6 matmuls back-to-back,
                halving the arrival bandwidth the PE needs during the
                HBM-bound startup."""
                msum, mr, mi = movs
                # w_ is ordered (wr, w3, w2) to match pss (psK, psR, psI):
                # psI<-w2 consumes mr, psR<-w3 consumes mi, psK<-wr the sum
                for k in range(KT):
                    for sel, mv in ((2, mr), (1, mi), (0, msum)):
                        for w_, pss in wjobs:
                            nc.tensor.matmul(
                                pss[sel][:], w_[sel][:, k * P:(k + 1) * P],
                                mv(k)[:],
                                start=(side == 0 and k == 0),
                                stop=(side == 1 and k == KT - 1))

            # startup emission order: m=0 x-side weights (sync DMA +
            # DVE combos) FIRST so the combos are not stuck in the DVE
            # FIFO behind h-dependent sum ops, then the x stream, the
            # rec weights, and last the h stream.  The two wr tiles load
            # before the wi tiles so the psK chains (which need only wr
            # and the x sums) can start ~2.5us in, before the combos.
            wrs0, wis0, wx0 = [], [], []
            for g in range(2):
                wr = wts.tile([P, MCOLS], F16, tag="w", name=f"wr0{g}")
                nc.sync.dma_start(wr[:], w1[0, g, 0])
                wrs0.append(wr)
            for g in range(2):
                wi = wts.tile([P, MCOLS], F16, tag="w", name=f"wi0{g}")
                nc.sync.dma_start(wi[:], w1[0, g, 1])
                wis0.append(wi)
            for g in range(2):
                w2t = wts.tile([P, MCOLS], F16, tag="w", name=f"w2t0{g}")
                nc.vector.tensor_tensor(w2t[:], wrs0[g][:], wis0[g][:], OP.add)
                w3t = wts.tile([P, MCOLS], F16, tag="w", name=f"w3t0{g}")
                nc.vector.tensor_tensor(w3t[:], wis0[g][:], wrs0[g][:],
                                        OP.subtract)
                wx0.append((wrs0[g], w2t, w3t))
            stream_acts(xT, xs, xsm)
            wrecs0 = [load_pair(w1[0, g, 2], w1[0, g, 3]) for g in range(2)]
            stream_acts(hT, hs, hsm)
            w2m0 = None

            # ---- phase 1: z and r gates, rh = clip(r)*h ----
            for m in range(MT):
                if m == MT - 1:
                    # hoist phase-2 m=0 weights ahead of the phase-1 tail
                    # so its combos clear the DVE before the phase switch
                    w2m0 = (load_pair(w2[0, 0], w2[0, 1]),
                            load_pair(w2[0, 2], w2[0, 3]))
                if m == 0:
                    jobs = [(g, wx0[g], job_psums()) for g in range(2)]
                    # x-side chain-major: psK chains only need wr + xsum
                    # (first bytes to arrive); psR/psI run once the wi
                    # tiles and DVE combos land
                    for g, sel, mv in ((0, 0, xsu), (1, 0, xsu),
                                       (0, 1, xi), (0, 2, xr),
                                       (1, 1, xi), (1, 2, xr)):
                        _, wx, pss = jobs[g]
                        w_ = (wx[0], wx[2], wx[1])
                        for k in range(KT):
                            nc.tensor.matmul(
                                pss[sel][:],
                                w_[sel][:, k * P:(k + 1) * P], mv(k)[:],
                                start=(k == 0), stop=False)
                    accum_fused([((wrecs0[g][0], wrecs0[g][2], wrecs0[g][1]),
                                  pss) for g, wx, pss in jobs], movh, 1)
                    for g, wx, pss in jobs:
                        combine_zr(g, m, pss)
                else:
                    for g in range(2):
                        wx = load_pair(w1[m, g, 0], w1[m, g, 1])
                        wrec = load_pair(w1[m, g, 2], w1[m, g, 3])
                        pss = job_psums()
                        accum_side(wx, movx, pss, 0)
                        accum_side(wrec, movh, pss, 1)
                        combine_zr(g, m, pss)

            # tail prep: zh/mu for the last m-tile (runs in late phase 1)
            for half in range(2):
                mm = (MT - 1) + half * MT
                nc.vector.scalar_tensor_tensor(
                    zh[:, half, :], zs[:, mm, :], 1.0, hs[:, mm, :],
                    op0=OP.min, op1=OP.mult)              # min(z,1)*h
                nc.vector.scalar_tensor_tensor(
                    mu[:, half, :], zs[:, mm, :], 1.0, ones[:],
                    op0=OP.min, op1=OP.subtract)          # min(z,1)-1

            # ---- phase 2: hh gate + final combine ----
            for m in range(MT):
                if m == 0:
                    wx, wrec = w2m0
                else:
                    wx = load_pair(w2[m, 0], w2[m, 1])
                    wrec = load_pair(w2[m, 2], w2[m, 3])
                pss = job_psums()
                if m == MT - 1:
                    # last job in column halves so only a half-width
                    # combine chain trails the final matmul
                    for cols in (slice(0, N // 2), slice(N // 2, N)):
                        accum_side(wx, movx, pss, 0, cols=cols)
                        accum_side(wrec, movrh, pss, 1, cols=cols)
                        combine_h_last(m, pss, cols)
                else:
                    accum_side(wx, movx, pss, 0)
                    accum_side(wrec, movrh, pss, 1)
                    combine_h(m, pss, slice(0, N))

    nc.compile()
    return nc


def _tiles(mat):
    # (1024, 1024) -> [p, k, 128, 128] tile array
    return mat.reshape(KT, P, MT, P).transpose(2, 0, 1, 3)


def _gate_blob(mats):
    """[p, 4, 128, MCOLS] fp16 weight blob from (R, I, RR, IR)."""
    arr = np.stack([_tiles(m) for m in mats])  # [4, p, k, part, col]
    arr = arr.transpose(1, 0, 3, 2, 4)         # [p, mat, part, k, col]
    return arr.reshape(MT, 4, P, MCOLS).astype(np.float16)


def prepare_in_maps(inputs, h_tm1, real_kernel, imaginary_kernel,
                    real_recurrent_kernel, imaginary_recurrent_kernel,
                    real_bias, imaginary_bias):
    inputs = np.asarray(inputs, dtype=np.float32)
    h_tm1 = np.asarray(h_tm1, dtype=np.float32)

    def gate(Wmat, g):
        return np.asarray(Wmat[:, g * U:(g + 1) * U], dtype=np.float32)

    def mats(g):
        R, I = gate(real_kernel, g), gate(imaginary_kernel, g)
        RR, IR = gate(real_recurrent_kernel, g), gate(imaginary_recurrent_kernel, g)
        return (R, I, RR, IR)

    w1_np = np.ascontiguousarray(
        np.stack([_gate_blob(mats(0)), _gate_blob(mats(1))], axis=1))
    w2_np = np.ascontiguousarray(_gate_blob(mats(2)))

    def cat_bias(g):
        return np.concatenate([
            np.asarray(real_bias[g * U:(g + 1) * U], dtype=np.float32),
            np.asarray(imaginary_bias[g * U:(g + 1) * U], dtype=np.float32),
        ])

    bzr_np = np.ascontiguousarray(np.stack(
        [0.2 * cat_bias(g) + 0.5 for g in range(2)]).reshape(2, 2 * MT, P)
        .transpose(2, 0, 1))
    bh_np = np.ascontiguousarray(cat_bias(2).reshape(2 * MT, P).T)

    in_maps = []
    for c in range(NCORES):
        sl = slice(c * N, (c + 1) * N)
        in_maps.append({
            "xT": inputs[sl].T.astype(np.float16),
            "hT": h_tm1[sl].T.astype(np.float16),
            "w1": w1_np, "w2": w2_np, "bzr": bzr_np, "bh": bh_np,
        })
    return in_maps


def get_nc():
    if "nc" not in _CACHE:
        _CACHE["nc"] = _build()
    return _CACHE["nc"]


def gather(results):
    out = np.empty((B, F), dtype=np.float32)
    for c in range(NCORES):
        out[c * N:(c + 1) * N] = res_oT(results, c)
    return out


def res_oT(results, c):
    return results[c]["oT"].T


def kernel(**inputs):
    nc = get_nc()
    in_maps = prepare_in_maps(**inputs)
    res = run_bass_kernel_spmd(nc, in_maps, list(range(NCORES)))
    return gather(res.results)


# revision 22
# speedup vs baseline: 1.0187x; 1.0007x over previous
"""CGRU cell on 8 Trainium2 NeuronCores.

Strategy: data-parallel over the batch dim (4096 -> 8 x 512). Each core
computes its h-shard with zero cross-core communication; weights are
replicated.

On-core compute runs in transposed space ([feature, batch]): the gate
pre-activations are (x @ W)^T = W^T @ x^T, so W tiles are the stationary
matmul operand and x^T/h^T tiles [128, 512] are the moving operand
(batch = 512 = one full fp32 PSUM bank).

The complex product uses Karatsuba (3 real matmuls instead of 4):
  real = k1 + xi@(I-R),  imag = k1 - xr@(R+I),  k1 = (xr+xi)@R
so each (gate, m-tile) job runs three 16-matmul PSUM chains (k1 /
real-extra / imag-extra over the input and recurrent halves) = 48
matmuls instead of the naive 64 -- a 25% cut in PE streaming time,
which is the bottleneck.  The combined weights R+I and I-R are built
on-chip from the streamed R, I tiles with one scalar_tensor_tensor
each, so weight DMA stays at 4 tiles/job.  The activation sums xr+xi,
hr+hi (and clip(r)*h sums for the candidate gate) are computed on-chip
on the vector engine.  Matmuls are fp16 with fp32 PSUM accumulation;
PSUM pairs are combined (k1 + extra) on the vector engine, activations
on the scalar engine.

DMA queues: weights stream on the sync queue, activations load on the
gpsimd queue, outputs store (fp16, upcast on host) on the sync queue.
"""

import numpy as np

import concourse.bass as bass
import concourse.mybir as mybir
import concourse.tile as tile
from concourse import bacc
from concourse.bass_utils import run_bass_kernel_spmd

B, D, U = 4096, 1024, 1024
NCORES = 8
N = B // NCORES          # batch rows per core (moving free dim)
P = 128                  # partition size
KT = D // P              # 8 k-tiles per complex half
MT = U // P              # 8 m-tiles per complex half
F = 2 * U                # 2048 features
MCOLS = KT * P           # 1024 cols per per-matrix weight tile

F16 = mybir.dt.float16
F32 = mybir.dt.float32
AF = mybir.ActivationFunctionType
OP = mybir.AluOpType

_CACHE = {}


def _build():
    nc = bacc.Bacc("TRN2", target_bir_lowering=False, debug=False)

    xT = nc.dram_tensor("xT", [F, N], F16, kind="ExternalInput")
    hT = nc.dram_tensor("hT", [F, N], F16, kind="ExternalInput")
    w1 = nc.dram_tensor("w1", [MT, 2, 4, P, MCOLS], F16, kind="ExternalInput")
    w2 = nc.dram_tensor("w2", [MT, 4, P, MCOLS], F16, kind="ExternalInput")
    bzr = nc.dram_tensor("bzr", [P, 2, 2 * MT], F32, kind="ExternalInput")
    bh = nc.dram_tensor("bh", [P, 2 * MT], F32, kind="ExternalInput")
    oT = nc.dram_tensor("oT", [F, N], F16, kind="ExternalOutput")

    with tile.TileContext(nc) as tc:
        with (
            tc.tile_pool(name="res", bufs=1) as res,
            tc.tile_pool(name="wts", bufs=24) as wts,
            tc.tile_pool(name="act", bufs=10) as act,
            tc.tile_pool(name="ps", bufs=8, space="PSUM") as psp,
        ):
            # PE warmup: dummy matmuls on a zeroed tile keep the HAM
            # activity window busy while the first real DMAs land.
            wsrc = res.tile([P, P], F16, tag="wsrc")
            dmov = res.tile([P, N], F16, tag="dmov")
            nc.vector.memset(wsrc[:], 0.0)
            nc.vector.memset(dmov[:], 0.0)
            wps = psp.tile([P, N], F32, tag="ps")
            for _ in range(8):
                nc.tensor.matmul(wps[:], wsrc[:], dmov[:], start=True, stop=True)

            xs = res.tile([P, 2 * MT, N], F16, tag="xs")
            xsm = res.tile([P, MT, N], F16, tag="xsm")
            hs = res.tile([P, 2 * MT, N], F16, tag="hs")
            hsm = res.tile([P, MT, N], F16, tag="hsm")
            zs = res.tile([P, 2 * MT, N], F16, tag="zs")
            rh = res.tile([P, 2 * MT, N], F16, tag="rh")
            rhsm = res.tile([P, MT, N], F16, tag="rhsm")
            zh = res.tile([P, 2, N], F16, tag="zh")
            mu = res.tile([P, 2, N], F16, tag="mu")
            ones = res.tile([P, N], F16, tag="ones")
            nc.vector.memset(ones[:], 1.0)
            bz_sb = res.tile([P, 2, 2 * MT], F32, tag="bz")
            bh_sb = res.tile([P, 2 * MT], F32, tag="bh")

            nc.scalar.dma_start(bz_sb[:], bzr[:])
            nc.scalar.dma_start(bh_sb[:], bh[:])

            def load_pair(src_r, src_i):
                """Stream R, I; build the Karatsuba combos on the DVE."""
                wr = wts.tile([P, MCOLS], F16, tag="w")
                nc.sync.dma_start(wr[:], src_r)
                wi = wts.tile([P, MCOLS], F16, tag="w")
                nc.sync.dma_start(wi[:], src_i)
                w2t = wts.tile([P, MCOLS], F16, tag="w")    # R+I
                nc.vector.tensor_tensor(w2t[:], wr[:], wi[:], OP.add)
                w3t = wts.tile([P, MCOLS], F16, tag="w")    # I-R
                nc.vector.tensor_tensor(w3t[:], wi[:], wr[:], OP.subtract)
                return wr, w2t, w3t

            def stream_acts(src_dram, dst, dst_sum):
                """(real, imag) pairs in consumption order; the on-chip
                sum follows each pair so the k1 chain is fed at arrival
                rate."""
                for k in range(MT):
                    nc.gpsimd.dma_start(dst[:, k, :],
                                        src_dram[k * P:(k + 1) * P, :])
                    nc.gpsimd.dma_start(dst[:, MT + k, :],
                                        src_dram[(MT + k) * P:(MT + k + 1) * P, :])
                    nc.vector.tensor_tensor(dst_sum[:, k, :], dst[:, k, :],
                                            dst[:, MT + k, :], OP.add)

            def accum_side(w_, movs, pss, side, kmajor=False,
                           cols=slice(0, N), k_first=False):
                """24 matmuls: one (input or recurrent) side of a job.

                kmajor interleaves the three chains per k-tile so PE
                consumption of freshly-DMA'd moving tiles matches
                arrival order (xr, xi, then the DVE'd sum).  k_first
                stops the psK chain earliest so the k1 copy and the
                real-half combine overlap the trailing psR/psI chains
                (used for the final job to shorten the kernel tail)."""
                wr_, w2_, w3_ = w_
                msum, mr, mi = movs
                psK, psR, psI = pss
                if k_first:
                    triples = ((psK, wr_, msum), (psR, w3_, mi),
                               (psI, w2_, mr))
                else:
                    triples = ((psI, w2_, mr), (psR, w3_, mi),
                               (psK, wr_, msum))

                def mm(ps, w, mv, k):
                    nc.tensor.matmul(ps[:, cols], w[:, k * P:(k + 1) * P],
                                     mv(k)[:, cols],
                                     start=(side == 0 and k == 0),
                                     stop=(side == 1 and k == KT - 1))

                if kmajor:
                    for k in range(KT):
                        for ps, w, mv in triples:
                            mm(ps, w, mv, k)
                else:
                    for ps, w, mv in triples:
                        for k in range(KT):
                            mm(ps, w, mv, k)

            def xr(k):
                return xs[:, k, :]

            def xi(k):
                return xs[:, MT + k, :]

            def xsu(k):
                return xsm[:, k, :]

            def hr(k):
                return hs[:, k, :]

            def hi(k):
                return hs[:, MT + k, :]

            def hsu(k):
                return hsm[:, k, :]

            def rhr(k):
                return rh[:, k, :]

            def rhi(k):
                return rh[:, MT + k, :]

            def rhsu(k):
                return rhsm[:, k, :]

            movx = (xsu, xr, xi)
            movh = (hsu, hr, hi)
            movrh = (rhsu, rhr, rhi)

            def combine_zr(g, m, pss):
                psK, psR, psI = pss
                # DVE can read only one PSUM input per instruction, so the
                # shared k1 term goes through SBUF once (on the scalar eng)
                k1s = act.tile([P, N], F16, tag="k1s")
                nc.scalar.activation(k1s[:], psK[:], AF.Copy)
                for half, psX in ((0, psR), (1, psI)):
                    mm = m if half == 0 else MT + m
                    d = act.tile([P, N], F16, tag="d")
                    nc.vector.tensor_tensor(d[:], k1s[:], psX[:],
                                            OP.add if half == 0 else OP.subtract)
                    if g == 0:
                        nc.scalar.activation(zs[:, mm, :], d[:], AF.Relu,
                                             bias=bz_sb[:, 0, mm:mm + 1],
                                             scale=0.2)
                    else:
                        rr = act.tile([P, N], F16, tag="rr")
                        nc.scalar.activation(rr[:], d[:], AF.Relu,
                                             bias=bz_sb[:, 1, mm:mm + 1],
                                             scale=0.2)
                        nc.vector.scalar_tensor_tensor(
                            rh[:, mm, :], rr[:], 1.0, hs[:, mm, :],
                            op0=OP.min, op1=OP.mult)
                if g == 1:
                    nc.vector.tensor_tensor(rhsm[:, m, :], rh[:, m, :],
                                            rh[:, MT + m, :], OP.add)

            def combine_h(m, pss, cols):
                psK, psR, psI = pss
                k1s = act.tile([P, N], F16, tag="k1s")
                nc.scalar.activation(k1s[:, cols], psK[:, cols], AF.Copy)
                for half, psX in ((0, psR), (1, psI)):
                    mm = m if half == 0 else MT + m
                    d = act.tile([P, N], F16, tag="d")
                    nc.vector.tensor_tensor(d[:, cols], k1s[:, cols],
                                            psX[:, cols],
                                            OP.add if half == 0 else OP.subtract)
                    t = act.tile([P, N], F16, tag="t")
                    nc.scalar.activation(t[:, cols], d[:, cols], AF.Tanh,
                                         bias=bh_sb[:, mm:mm + 1])
                    e = act.tile([P, N], F16, tag="d")
                    nc.vector.scalar_tensor_tensor(
                        e[:, cols], t[:, cols], -1.0, hs[:, mm, cols],
                        op0=OP.mult, op1=OP.add)          # h - t
                    f = act.tile([P, N], F16, tag="d")
                    nc.vector.scalar_tensor_tensor(
                        f[:, cols], zs[:, mm, cols], 1.0, e[:, cols],
                        op0=OP.min, op1=OP.mult)          # min(z,1)*(h-t)
                    o = act.tile([P, N], F16, tag="o")
                    nc.vector.tensor_tensor(o[:, cols], f[:, cols],
                                            t[:, cols], OP.add)
                    nc.sync.dma_start(oT[mm * P:(mm + 1) * P, cols],
                                        o[:, cols])

            def combine_h_last(m, pss, cols):
                """Tail-optimized combine: zh = min(z,1)*h and
                mu = min(z,1)-1 are precomputed, so only two DVE ops
                trail the tanh:  o = zh - mu*t."""
                psK, psR, psI = pss
                k1s = act.tile([P, N], F16, tag="k1s")
                nc.scalar.activation(k1s[:, cols], psK[:, cols], AF.Copy)
                for half, psX in ((0, psR), (1, psI)):
                    mm = m if half == 0 else MT + m
                    d = act.tile([P, N], F16, tag="d")
                    nc.vector.tensor_tensor(d[:, cols], k1s[:, cols],
                                            psX[:, cols],
                                            OP.add if half == 0 else OP.subtract)
                    t = act.tile([P, N], F16, tag="t")
                    nc.scalar.activation(t[:, cols], d[:, cols], AF.Tanh,
                                         bias=bh_sb[:, mm:mm + 1])
                    v = act.tile([P, N], F16, tag="d")
                    nc.vector.tensor_tensor(v[:, cols], mu[:, half, cols],
                                            t[:, cols], OP.mult)
                    o = act.tile([P, N], F16, tag="o")
                    nc.vector.tensor_tensor(o[:, cols], zh[:, half, cols],
                                            v[:, cols], OP.subtract)
                    nc.sync.dma_start(oT[mm * P:(mm + 1) * P, cols],
                                        o[:, cols])

            def job_psums():
                return tuple(psp.tile([P, N], F32, tag="ps", name=f"ps{i}")
                             for i in range(3))

            def accum_fused(wjobs, movs, side):
                """Both m=0 jobs' chains interleaved per k-tile: each
                freshly-DMA'd moving tile feeds 6 matmuls back-to-back,
                halving the arrival bandwidth the PE needs during the
                HBM-bound startup."""
                msum, mr, mi = movs
                # w_ is ordered (wr, w3, w2) to match pss (psK, psR, psI):
                # psI<-w2 consumes mr, psR<-w3 consumes mi, psK<-wr the sum
                for k in range(KT):
                    for sel, mv in ((2, mr), (1, mi), (0, msum)):
                        for w_, pss in wjobs:
                            nc.tensor.matmul(
                                pss[sel][:], w_[sel][:, k * P:(k + 1) * P],
                                mv(k)[:],
                                start=(side == 0 and k == 0),
                                stop=(side == 1 and k == KT - 1))

            # startup emission order: m=0 x-side weights (sync DMA +
            # DVE combos) FIRST so the combos are not stuck in the DVE
            # FIFO behind h-dependent sum ops, then the x stream, the
            # rec weights, and last the h stream.  The two wr tiles load
            # before the wi tiles so the psK chains (which need only wr
            # and the x sums) can start ~2.5us in, before the combos.
            wrs0, wis0, wx0 = [], [], []
            for g in range(2):
                wr = wts.tile([P, MCOLS], F16, tag="w", name=f"wr0{g}")
                nc.sync.dma_start(wr[:], w1[0, g, 0])
                wrs0.append(wr)
            for g in range(2):
                wi = wts.tile([P, MCOLS], F16, tag="w", name=f"wi0{g}")
                nc.sync.dma_start(wi[:], w1[0, g, 1])
                wis0.append(wi)
            for g in range(2):
                w2t = wts.tile([P, MCOLS], F16, tag="w", name=f"w2t0{g}")
                nc.vector.tensor_tensor(w2t[:], wrs0[g][:], wis0[g][:], OP.add)
                w3t = wts.tile([P, MCOLS], F16, tag="w", name=f"w3t0{g}")
                nc.vector.tensor_tensor(w3t[:], wis0[g][:], wrs0[g][:],
                                        OP.subtract)
                wx0.append((wrs0[g], w2t, w3t))
            stream_acts(xT, xs, xsm)
            wrecs0 = [load_pair(w1[0, g, 2], w1[0, g, 3]) for g in range(2)]
            stream_acts(hT, hs, hsm)
            w2m0 = None

            # ---- phase 1: z and r gates, rh = clip(r)*h ----
            for m in range(MT):
                if m == MT - 1:
                    # hoist phase-2 m=0 weights ahead of the phase-1 tail
                    # so its combos clear the DVE before the phase switch
                    w2m0 = (load_pair(w2[0, 0], w2[0, 1]),
                            load_pair(w2[0, 2], w2[0, 3]))
                if m == 0:
                    jobs = [(g, wx0[g], job_psums()) for g in range(2)]
                    # x-side chain-major: psK chains only need wr + xsum
                    # (first bytes to arrive); psR/psI run once the wi
                    # tiles and DVE combos land
                    for g, sel, mv in ((0, 0, xsu), (1, 0, xsu),
                                       (0, 1, xi), (0, 2, xr),
                                       (1, 1, xi), (1, 2, xr)):
                        _, wx, pss = jobs[g]
                        w_ = (wx[0], wx[2], wx[1])
                        for k in range(KT):
                            nc.tensor.matmul(
                                pss[sel][:],
                                w_[sel][:, k * P:(k + 1) * P], mv(k)[:],
                                start=(k == 0), stop=False)
                    accum_fused([((wrecs0[g][0], wrecs0[g][2], wrecs0[g][1]),
                                  pss) for g, wx, pss in jobs], movh, 1)
                    for g, wx, pss in jobs:
                        combine_zr(g, m, pss)
                else:
                    for g in range(2):
                        wx = load_pair(w1[m, g, 0], w1[m, g, 1])
                        wrec = load_pair(w1[m, g, 2], w1[m, g, 3])
                        pss = job_psums()
                        accum_side(wx, movx, pss, 0)
                        accum_side(wrec, movh, pss, 1)
                        combine_zr(g, m, pss)

            # tail prep: zh/mu for the last m-tile (runs in late phase 1)
            for half in range(2):
                mm = (MT - 1) + half * MT
                nc.vector.scalar_tensor_tensor(
                    zh[:, half, :], zs[:, mm, :], 1.0, hs[:, mm, :],
                    op0=OP.min, op1=OP.mult)              # min(z,1)*h
                nc.vector.scalar_tensor_tensor(
                    mu[:, half, :], zs[:, mm, :], 1.0, ones[:],
                    op0=OP.min, op1=OP.subtract)          # min(z,1)-1

            # ---- phase 2: hh gate + final combine ----
            for m in range(MT):
                if m == 0:
                    wx, wrec = w2m0
                else:
                    wx = load_pair(w2[m, 0], w2[m, 1])
                    wrec = load_pair(w2[m, 2], w2[m, 3])
                pss = job_psums()
                if m == MT - 1:
                    # last job in column halves so only a half-width
                    # combine chain trails the final matmul
                    for cols in (slice(0, N // 2), slice(N // 2, N)):
                        accum_side(wx, movx, pss, 0, cols=cols)
                        accum_side(wrec, movrh, pss, 1, cols=cols,
                                   k_first=True)
                        combine_h_last(m, pss, cols)
                else:
                    accum_side(wx, movx, pss, 0)
                    accum_side(wrec, movrh, pss, 1)
                    combine_h(m, pss, slice(0, N))

    nc.compile()
    return nc


def _tiles(mat):
    # (1024, 1024) -> [p, k, 128, 128] tile array
    return mat.reshape(KT, P, MT, P).transpose(2, 0, 1, 3)


def _gate_blob(mats):
    """[p, 4, 128, MCOLS] fp16 weight blob from (R, I, RR, IR)."""
    arr = np.stack([_tiles(m) for m in mats])  # [4, p, k, part, col]
    arr = arr.transpose(1, 0, 3, 2, 4)         # [p, mat, part, k, col]
    return arr.reshape(MT, 4, P, MCOLS).astype(np.float16)


def prepare_in_maps(inputs, h_tm1, real_kernel, imaginary_kernel,
                    real_recurrent_kernel, imaginary_recurrent_kernel,
                    real_bias, imaginary_bias):
    inputs = np.asarray(inputs, dtype=np.float32)
    h_tm1 = np.asarray(h_tm1, dtype=np.float32)

    def gate(Wmat, g):
        return np.asarray(Wmat[:, g * U:(g + 1) * U], dtype=np.float32)

    def mats(g):
        R, I = gate(real_kernel, g), gate(imaginary_kernel, g)
        RR, IR = gate(real_recurrent_kernel, g), gate(imaginary_recurrent_kernel, g)
        return (R, I, RR, IR)

    w1_np = np.ascontiguousarray(
        np.stack([_gate_blob(mats(0)), _gate_blob(mats(1))], axis=1))
    w2_np = np.ascontiguousarray(_gate_blob(mats(2)))

    def cat_bias(g):
        return np.concatenate([
            np.asarray(real_bias[g * U:(g + 1) * U], dtype=np.float32),
            np.asarray(imaginary_bias[g * U:(g + 1) * U], dtype=np.float32),
        ])

    bzr_np = np.ascontiguousarray(np.stack(
        [0.2 * cat_bias(g) + 0.5 for g in range(2)]).reshape(2, 2 * MT, P)
        .transpose(2, 0, 1))
    bh_np = np.ascontiguousarray(cat_bias(2).reshape(2 * MT, P).T)

    in_maps = []
    for c in range(NCORES):
        sl = slice(c * N, (c + 1) * N)
        in_maps.append({
            "xT": inputs[sl].T.astype(np.float16),
            "hT": h_tm1[sl].T.astype(np.float16),
            "w1": w1_np, "w2": w2_np, "bzr": bzr_np, "bh": bh_np,
        })
    return in_maps


def get_nc():
    if "nc" not in _CACHE:
        _CACHE["nc"] = _build()
    return _CACHE["nc"]


def gather(results):
    out = np.empty((B, F), dtype=np.float32)
    for c in range(NCORES):
        out[c * N:(c + 1) * N] = res_oT(results, c)
    return out


def res_oT(results, c):
    return results[c]["oT"].T


def kernel(**inputs):
    nc = get_nc()
    in_maps = prepare_in_maps(**inputs)
    res = run_bass_kernel_spmd(nc, in_maps, list(range(NCORES)))
    return gather(res.results)
